# revision 28
# baseline (speedup 1.0000x reference)
"""InterleavedHeadAttention Trainium2 kernel.

Sharding (8 cores): core c handles batch b = c//4 and 4 output heads
[4*(c%4), 4*(c%4)+4).  The alpha head-mixing einsum is folded into the
QKV projection weights on the host, so each core's projections only
produce its own heads' (h, p, d) slices.  The pseudo-head merge uses
(p, n) flat ordering internally (attention is permutation invariant;
the token-causal mask depends only on n), which makes every layout a
direct view of a matmul output.  collapse and the 1/den softmax
normalization are applied on-device; Wo is folded with collapse and
applied per-head, each core emitting a partial (S, HID) bf16 output
that the host sums in f32 (+bo).

All per-core inputs are packed into a single 1-D bf16 "blob" tensor:
per-exec dispatch overhead in the PJRT/axon path scales with the number
of bound buffers, so 17 inputs -> 1 input is a large wall-clock win.

Compute structure (v2):
- Scores for one (Jn, pk) key block land in a 2-bank PSUM macro-tile
  [128, 1024] = [pq0 | pq1]; one Exp activation covers both halves.
  Score matmuls run full 512-wide even in diagonal blocks (the masked
  columns hold real, small q.k values); the AV matmuls only consume the
  causally-allowed column range, and the 128-wide diagonal sub-block is
  tri-masked on the probs.
- Softmax denominators are accumulated via an extra all-ones row in the
  V tiles (vaug); per query window the 8 (h, pq) denominator rows are
  copied into one [8, 512] tile, inverted with a single batched
  reciprocal, and broadcast back to 64 rows with a selector matmul.
- Loop order is window-outer (In), head-inner, so the output projection
  for window 0 overlaps the attention of window 1.
- Q/K biases are folded into the PSUM->SBUF eviction as per-partition
  tensor_scalar adds; V bias keeps the rank-1 ones-row matmul.
"""
import numpy as np
import ml_dtypes

import concourse.bacc as bacc
import concourse.bass as bass
import concourse.tile as tile
import concourse.mybir as mybir
from concourse.bass_utils import run_bass_kernel_spmd

B, S, HID, H, P = 2, 1024, 1024, 16, 2
D = HID // H          # 64
HL = 4                # heads per core
G = HL * P            # (h,p) groups per core = 8
HPD = HL * P * D      # 512 projection rows per core
BF = mybir.dt.bfloat16
F32 = mybir.dt.float32
NCORES = 8
KT = HID // 128       # 8 k tiles over hidden
NT = S // 512         # 2 query windows

# blob layout (bf16 element offsets)
OFF_XT = 0                            # (HID, S)
OFF_WQ = OFF_XT + HID * S             # (HID, HPD) each
OFF_WK = OFF_WQ + HID * HPD
OFF_WV = OFF_WK + HID * HPD
OFF_BQK = OFF_WV + HID * HPD          # (128, 12): bqT | bkT | bk2T
OFF_BV = OFF_BQK + 128 * 12           # (512,)
OFF_WO = OFF_BV + HPD                 # (HL, P*D, HID)
OFF_TRI = OFF_WO + HL * P * D * HID   # (128, 128)
TOT = OFF_TRI + 128 * 128

_compiled = None


def _build():
    nc = bacc.Bacc()
    blob = nc.dram_tensor("blob", (TOT,), BF, kind="ExternalInput")
    out = nc.dram_tensor("o", (S, HID), BF, kind="ExternalOutput")

    def bview(off, p, e):
        return blob[off:off + p * e].rearrange("(p e) -> p e", e=e)

    with tile.TileContext(nc) as tc:
        with tc.tile_pool(name="persist", bufs=1) as pp, \
             tc.tile_pool(name="ppool", bufs=6) as ppl, \
             tc.tile_pool(name="small", bufs=4) as sml, \
             tc.tile_pool(name="osb", bufs=3) as osb, \
             tc.tile_pool(name="psc", bufs=2, space=bass.MemorySpace.PSUM) as psc, \
             tc.tile_pool(name="ps1", bufs=4, space=bass.MemorySpace.PSUM) as ps1:

            ones = pp.tile([1, 512], BF, tag="ones", name="ones")
            nc.gpsimd.memset(ones[:], 1.0)
            ones4 = pp.tile([128, 64], BF, tag="ones4", name="ones4")
            nc.gpsimd.memset(ones4[:], 1.0)

            # Consolidated loads: few large DMAs with 3-D access patterns,
            # ordered so the V projection (xt + wv) can start first.
            xt_big = pp.tile([128, KT * S], BF, tag="xt", name="xt")
            w_big = pp.tile([128, 3 * KT * HPD], BF, tag="w", name="w")

            def load_xt(half):
                nc.sync.dma_start(
                    xt_big[:, half * 4 * S:(half + 1) * 4 * S].rearrange(
                        "p (k n) -> p k n", n=S),
                    blob[OFF_XT + half * 4 * 128 * S:
                         OFF_XT + (half + 1) * 4 * 128 * S].rearrange(
                        "(k p n) -> p k n", k=4, p=128))

            def load_w(i):
                nc.sync.dma_start(
                    w_big[:, i * KT * HPD:(i + 1) * KT * HPD].rearrange(
                        "p (k c) -> p k c", c=HPD),
                    blob[OFF_WQ + i * HID * HPD:OFF_WQ + (i + 1) * HID * HPD]
                    .rearrange("(k p c) -> p k c", k=KT, p=128))

            load_xt(0)
            load_w(2)   # wv
            load_xt(1)
            load_w(0)   # wq
            load_w(1)   # wk
            xt_sb = [xt_big[:, k * S:(k + 1) * S] for k in range(KT)]
            w_sb = {nm: [w_big[:, (i * KT + k) * HPD:(i * KT + k + 1) * HPD]
                         for k in range(KT)]
                    for i, nm in enumerate(("q", "k", "v"))}

            tri_sb = pp.tile([128, 128], BF, tag="tri", name="tri")
            nc.sync.dma_start(tri_sb[:], bview(OFF_TRI, 128, 128))
            bqk_bf = pp.tile([128, 12], BF, tag="bqkb", name="bqkb")
            nc.sync.dma_start(bqk_bf[:], bview(OFF_BQK, 128, 12))
            bqk_sb = pp.tile([128, 12], F32, tag="bqk", name="bqk")
            nc.vector.tensor_copy(bqk_sb[:], bqk_bf[:])
            bv_sb = pp.tile([1, HPD], BF, tag="bv", name="bv")
            nc.sync.dma_start(bv_sb[:], bview(OFF_BV, 1, HPD))
            woe_big = pp.tile([128, HL * HID], BF, tag="woe", name="woe")
            nc.sync.dma_start(
                woe_big.rearrange("p (h j) -> p h j", j=HID),
                blob[OFF_WO:OFF_WO + HL * P * D * HID].rearrange(
                    "(h p j) -> p h j", h=HL, p=128))
            woe_sb = [woe_big[:, h * HID:(h + 1) * HID] for h in range(HL)]

            qt_sb = [pp.tile([128, S], BF, tag=f"qt{h}", name=f"qt{h}") for h in range(HL)]
            kt_sb = [pp.tile([128, S], BF, tag=f"kt{h}", name=f"kt{h}") for h in range(HL)]
            kt2_sb = [pp.tile([128, S], BF, tag=f"kt2{h}", name=f"kt2{h}") for h in range(HL)]
            vaug = [pp.tile([128, G * 65], BF, tag=f"va{j}", name=f"va{j}") for j in range(S // 128)]
            ot2 = [pp.tile([128, S], BF, tag=f"ot2{h}", name=f"ot2{h}") for h in range(HL)]
            oav = [[pp.tile([64, 512], BF, tag=f"oav{In}_{i}", name=f"oav{In}_{i}")
                    for i in range(G)] for In in range(NT)]
            # denominator rows live at partitions {0,32,64,96} of two tiles
            # (DVE writes must be 32-partition aligned); memset so the unused
            # lanes reciprocate 1.0 rather than garbage.
            den = [[pp.tile([128, 512], BF, tag=f"den{In}_{t}", name=f"den{In}_{t}")
                    for t in range(2)] for In in range(NT)]
            for In in range(NT):
                for t in range(2):
                    nc.gpsimd.memset(den[In][t][:], 1.0)

            def emit_qk(mt, nt):
                # Q and K projections for head mt, seq window nt.
                for nm in ("q", "k"):
                    bcol = 0 if nm == "q" else 4
                    acc = ps1.tile([128, 512], F32, tag="b1", name="acc")
                    for k in range(KT):
                        nc.tensor.matmul(
                            acc[:], w_sb[nm][k][:, mt * 128:(mt + 1) * 128],
                            xt_sb[k][:, nt * 512:(nt + 1) * 512],
                            start=(k == 0), stop=(k == KT - 1))
                    dst = qt_sb[mt] if nm == "q" else kt_sb[mt]
                    sl = slice(nt * 512, (nt + 1) * 512)
                    nc.vector.tensor_scalar_add(
                        dst[:, sl], acc[:], bqk_sb[:, bcol + mt:bcol + mt + 1])
                    if nm == "k":
                        # kt2 = kt with partition halves swapped; copy in
                        # SBUF (2x mode, and the PSUM acc is freed sooner)
                        nc.vector.tensor_copy(kt2_sb[mt][0:64, sl],
                                              kt_sb[mt][64:128, sl])
                        nc.vector.tensor_copy(kt2_sb[mt][64:128, sl],
                                              kt_sb[mt][0:64, sl])

            def emit_v(jt):
                # V projection for key block jt -> vaug (v columns + ones row).
                v3 = vaug[jt].rearrange("p (g e) -> p g e", e=65)
                nc.gpsimd.memset(v3[:, :, 64:65], 1.0)
                acc = ps1.tile([128, 512], F32, tag="b1", name="acc")
                for k in range(KT):
                    nc.tensor.matmul(
                        acc[:], xt_sb[k][:, jt * 128:(jt + 1) * 128],
                        w_sb["v"][k][:], start=(k == 0), stop=False)
                nc.tensor.matmul(acc[:], ones[:, 0:128], bv_sb[:],
                                 start=False, stop=True)
                nc.vector.tensor_copy(
                    v3[:, :, 0:64], acc[:].rearrange("p (g e) -> p g e", e=64))

            def emit_attn(In, h):
                JMAX = 4 * In + 4
                isl = slice(In * 512, (In + 1) * 512)
                avp = [ps1.tile([65, 512], F32, tag="b1", name="av") for _ in range(2)]
                for Jn in range(JMAX):
                    diag = Jn >= 4 * In
                    jsl = slice(Jn * 128, (Jn + 1) * 128)
                    if not diag:
                        # full-width block: macro = [pq0 | pq1] for one pk
                        for pk in range(2):
                            lhsA = (kt_sb[h] if pk == 0 else kt2_sb[h])
                            lhsB = (kt2_sb[h] if pk == 0 else kt_sb[h])
                            mac = psc.tile([128, 1024], F32, tag="sc", name="mac")
                            nc.tensor.matmul(
                                mac[:, 0:512], lhsA[0:64, jsl],
                                qt_sb[h][0:64, isl], start=True, stop=True)
                            nc.tensor.matmul(
                                mac[:, 512:1024], lhsB[64:128, jsl],
                                qt_sb[h][64:128, isl], start=True, stop=True)
                            pt = ppl.tile([128, 1024], BF, tag="p", name="p")
                            nc.scalar.activation(
                                pt[:], mac[:],
                                mybir.ActivationFunctionType.Exp, scale=0.125)
                            g = h * 2 + pk
                            first = (Jn == 0 and pk == 0)
                            nc.tensor.matmul(
                                avp[0][:, 0:512],
                                vaug[Jn][:, g * 65:g * 65 + 65],
                                pt[:, 0:512], start=first, stop=False)
                            nc.tensor.matmul(
                                avp[1][:, 0:512],
                                vaug[Jn][:, g * 65:g * 65 + 65],
                                pt[:, 512:1024], start=first, stop=False)
                        continue
                    # diagonal block: macro per pq packs the legal ranges of
                    # both pk contiguously: [c0:512] = pk0, [512:512+W] = pk1,
                    # so one exp covers exactly [c0:512+W].
                    c0 = 128 * (Jn - 4 * In)
                    W = 512 - c0
                    qsl = slice(In * 512 + c0, (In + 1) * 512)
                    for pq in range(2):
                        l0 = kt_sb[h] if pq == 0 else kt2_sb[h]    # k pseudo 0
                        l1 = kt2_sb[h] if pq == 0 else kt_sb[h]    # k pseudo 1
                        hp = slice(pq * 64, pq * 64 + 64)
                        mac = psc.tile([128, 1024], F32, tag="sc", name="mac")
                        nc.tensor.matmul(
                            mac[:, c0:512], l0[hp, jsl], qt_sb[h][hp, qsl],
                            start=True, stop=True)
                        nc.tensor.matmul(
                            mac[:, 512:512 + W], l1[hp, jsl], qt_sb[h][hp, qsl],
                            start=True, stop=True)
                        pt = ppl.tile([128, 1024], BF, tag="p", name="p")
                        nc.scalar.activation(
                            pt[:, c0:512 + W], mac[:, c0:512 + W],
                            mybir.ActivationFunctionType.Exp, scale=0.125)
                        nc.vector.tensor_mul(
                            pt[:, c0:c0 + 128], pt[:, c0:c0 + 128], tri_sb[:])
                        nc.vector.tensor_mul(
                            pt[:, 512:512 + 128], pt[:, 512:512 + 128], tri_sb[:])
                        g = h * 2
                        # both pk accumulate into avp[pq]
                        nc.tensor.matmul(
                            avp[pq][:, c0:512],
                            vaug[Jn][:, g * 65:g * 65 + 65],
                            pt[:, c0:512],
                            start=(In == 0 and Jn == 0), stop=False)
                        nc.tensor.matmul(
                            avp[pq][:, c0:512],
                            vaug[Jn][:, (g + 1) * 65:(g + 1) * 65 + 65],
                            pt[:, 512:512 + W],
                            start=False, stop=(Jn == JMAX - 1))
                for pq in range(2):
                    idx = h * 2 + pq
                    row = 32 * (idx % 4)
                    nc.vector.tensor_copy(
                        den[In][idx // 4][row:row + 1, :], avp[pq][64:65, :])
                    nc.vector.tensor_copy(oav[In][idx][:], avp[pq][0:64, :])

            def emit_norm(In, t):
                # reciprocal + normalize for (h, pq) pairs idx = 4t .. 4t+3
                rec = sml.tile([128, 512], BF, tag="rec", name="rec")
                with nc.allow_low_precision(reason="softmax recip bf16"):
                    nc.vector.reciprocal(rec[:], den[In][t][:])
                for idx in range(4 * t, 4 * t + 4):
                    h, pq = idx // 2, idx % 2
                    row = 32 * (idx % 4)
                    bcp = ps1.tile([64, 512], F32, tag="b1", name="bcp")
                    nc.tensor.matmul(
                        bcp[:], ones4[row:row + 1, :],
                        rec[row:row + 1, :], start=True, stop=True,
                        tile_position=(row, 0))
                    nc.vector.tensor_mul(
                        ot2[h][pq * 64:(pq + 1) * 64, In * 512:(In + 1) * 512],
                        oav[In][idx][:], bcp[:])

            def emit_outproj(In):
                for mt in range(4 * In, 4 * In + 4):
                    ob = osb.tile([128, 1024], BF, tag="ob", name="ob")
                    for jt in range(HID // 512):
                        op = ps1.tile([128, 512], F32, tag="b1", name="op")
                        for h in range(HL):
                            nc.tensor.matmul(
                                op[:], ot2[h][:, mt * 128:(mt + 1) * 128],
                                woe_sb[h][:, jt * 512:(jt + 1) * 512],
                                start=(h == 0), stop=(h == HL - 1))
                        if jt == 0:
                            nc.vector.tensor_copy(ob[:, 0:512], op[:])
                        else:
                            # ACT is idle during the out-projection tail
                            nc.scalar.copy(ob[:, 512:1024], op[:])
                    nc.sync.dma_start(out[mt * 128:(mt + 1) * 128, :], ob[:])

            # Interleaved emission: attention for a head starts as soon as its
            # q/k (window slice) and the first key blocks' v are ready, so the
            # ACT exp stream ramps early and projections fill PE stalls.
            for jt in range(4):
                emit_v(jt)
            emit_qk(0, 0)
            emit_qk(0, 1)
            emit_qk(1, 0)
            emit_qk(1, 1)
            emit_attn(0, 0)
            emit_qk(2, 0)
            emit_qk(2, 1)
            emit_attn(0, 1)
            emit_qk(3, 0)
            emit_qk(3, 1)
            emit_attn(0, 2)
            for jt in range(4, 8):
                emit_v(jt)
            emit_attn(0, 3)
            emit_attn(1, 0)
            emit_norm(0, 0)
            emit_norm(0, 1)
            emit_outproj(0)
            emit_attn(1, 1)
            emit_norm(1, 0)
            emit_attn(1, 2)
            emit_attn(1, 3)
            emit_norm(1, 1)
            emit_outproj(1)
    nc.compile()
    return nc


def _prep(inputs):
    bf = ml_dtypes.bfloat16
    hs = np.asarray(inputs["hidden_states"], np.float32)
    maps = []
    tri = np.triu(np.ones((128, 128), np.float32)).astype(bf)  # tri[r,c]=1 iff c>=r
    eff = {}
    for nm in ("q", "k", "v"):
        W = np.asarray(inputs[f"W{nm}"], np.float32)
        bb = np.asarray(inputs[f"b{nm}"], np.float32)
        al = np.asarray(inputs[f"alpha_{nm}"], np.float32)
        We = np.einsum("mhp,mdc->hpdc", al, W.reshape(H, D, HID))
        be = np.einsum("mhp,md->hpd", al, bb.reshape(H, D))
        eff[nm] = (We, be)
    Wo = np.asarray(inputs["Wo"], np.float32)
    col = np.asarray(inputs["collapse"], np.float32)
    Woe = np.einsum("hp,jhd->hpdj", col, Wo.reshape(HID, H, D))  # (H,P,D,HID)
    for c in range(NCORES):
        b, g = c // 4, c % 4
        hs_sl = slice(g * HL, (g + 1) * HL)
        parts = [np.ascontiguousarray(hs[b].T).astype(bf).reshape(-1)]
        for nm in ("q", "k", "v"):
            We, _ = eff[nm]
            Wslice = We[hs_sl].reshape(HPD, HID)      # (hpd, c)
            parts.append(np.ascontiguousarray(Wslice.T).astype(bf).reshape(-1))
        # (128, 12) per-partition bias columns: bqT | bkT | bk2T
        bq = eff["q"][1][hs_sl].reshape(HL, 128).T    # (128, HL)
        bk = eff["k"][1][hs_sl].reshape(HL, 128).T
        bk2 = np.concatenate([bk[64:128], bk[0:64]], axis=0)
        parts.append(np.concatenate([bq, bk, bk2], axis=1).astype(bf).reshape(-1))
        parts.append(eff["v"][1][hs_sl].reshape(-1).astype(bf))
        parts.append(Woe[hs_sl].reshape(-1).astype(bf))
        parts.append(tri.reshape(-1))
        blob = np.concatenate(parts)
        assert blob.shape[0] == TOT, blob.shape
        maps.append({"blob": blob})
    return maps


def kernel(**inputs):
    global _compiled
    if _compiled is None:
        _compiled = _build()
    maps = _prep(inputs)
    res = run_bass_kernel_spmd(_compiled, maps, core_ids=list(range(NCORES)))
    bo = np.asarray(inputs["bo"], np.float32)
    out = np.zeros((B, S, HID), np.float32)
    for c in range(NCORES):
        out[c // 4] += res.results[c]["o"].astype(np.float32)
    out += bo
    return out


# revision 30
# speedup vs baseline: 1.3824x; 1.3824x over previous
"""InterleavedHeadAttention Trainium2 kernel.

Sharding (8 cores): core c handles batch b = c//4 and 4 output heads
[4*(c%4), 4*(c%4)+4).  The alpha head-mixing einsum is folded into the
QKV projection weights on the host, so each core's projections only
produce its own heads' (h, p, d) slices.  The pseudo-head merge uses
(p, n) flat ordering internally (attention is permutation invariant;
the token-causal mask depends only on n), which makes every layout a
direct view of a matmul output.  collapse and the 1/den softmax
normalization are applied on-device; Wo is folded with collapse and
applied per-head, each core emitting a partial (S, HID) bf16 output
that the host sums in f32 (+bo).

All per-core inputs are packed into a single 1-D bf16 "blob" tensor:
per-exec dispatch overhead in the PJRT/axon path scales with the number
of bound buffers, so 17 inputs -> 1 input is a large wall-clock win.

Compute structure (v2):
- Scores for one (Jn, pk) key block land in a 2-bank PSUM macro-tile
  [128, 1024] = [pq0 | pq1]; one Exp activation covers both halves.
  Score matmuls run full 512-wide even in diagonal blocks (the masked
  columns hold real, small q.k values); the AV matmuls only consume the
  causally-allowed column range, and the 128-wide diagonal sub-block is
  tri-masked on the probs.
- Diagonal key blocks pack the legal (causal) column ranges of both key
  pseudo-heads contiguously per query pseudo-head, so the exp covers
  exactly the allowed region.
- Softmax denominators are accumulated via an extra all-ones row in the
  V tiles (vaug); per query window, denominator rows are parked at
  32-aligned partitions of a shared tile, inverted with one batched
  reciprocal per 4 rows, and row-broadcast with a K=1 ones matmul.
- Loop order is window-outer (In), head-inner, with projections
  interleaved between heads so the exp stream ramps early; the window-0
  output projection overlaps window-1 attention.
- Q/K biases are folded into the PSUM->SBUF eviction as per-partition
  tensor_scalar adds; V bias keeps the rank-1 ones-row matmul.
"""
import numpy as np
import ml_dtypes

import concourse.bacc as bacc
import concourse.bass as bass
import concourse.tile as tile
import concourse.mybir as mybir
from concourse.bass_utils import run_bass_kernel_spmd

B, S, HID, H, P = 2, 1024, 1024, 16, 2
D = HID // H          # 64
HL = 4                # heads per core
G = HL * P            # (h,p) groups per core = 8
HPD = HL * P * D      # 512 projection rows per core
BF = mybir.dt.bfloat16
F32 = mybir.dt.float32
NCORES = 8
KT = HID // 128       # 8 k tiles over hidden
NT = S // 512         # 2 query windows

# blob layout (bf16 element offsets)
OFF_XT = 0                            # (HID, S)
OFF_WQ = OFF_XT + HID * S             # (HID, HPD) each
OFF_WK = OFF_WQ + HID * HPD
OFF_WV = OFF_WK + HID * HPD
OFF_BQK = OFF_WV + HID * HPD          # (128, 12): bqT | bkT | bk2T
OFF_BV = OFF_BQK + 128 * 12           # (512,)
OFF_WO = OFF_BV + HPD                 # (HL, P*D, HID)
OFF_TRI = OFF_WO + HL * P * D * HID   # (128, 128)
TOT = OFF_TRI + 128 * 128

_compiled = None


def _build():
    nc = bacc.Bacc()
    blob = nc.dram_tensor("blob", (TOT,), BF, kind="ExternalInput")
    out = nc.dram_tensor("o", (S, HID), BF, kind="ExternalOutput")

    def bview(off, p, e):
        return blob[off:off + p * e].rearrange("(p e) -> p e", e=e)

    with tile.TileContext(nc) as tc:
        with tc.tile_pool(name="persist", bufs=1) as pp, \
             tc.tile_pool(name="ppool", bufs=6) as ppl, \
             tc.tile_pool(name="small", bufs=4) as sml, \
             tc.tile_pool(name="osb", bufs=3) as osb, \
             tc.tile_pool(name="psc", bufs=2, space=bass.MemorySpace.PSUM) as psc, \
             tc.tile_pool(name="ps1", bufs=4, space=bass.MemorySpace.PSUM) as ps1:

            ones = pp.tile([1, 512], BF, tag="ones", name="ones")
            nc.gpsimd.memset(ones[:], 1.0)
            ones4 = pp.tile([128, 64], BF, tag="ones4", name="ones4")
            nc.gpsimd.memset(ones4[:], 1.0)

            # Consolidated loads: few large DMAs with 3-D access patterns,
            # ordered so the V projection (xt + wv) can start first.
            xt_big = pp.tile([128, KT * S], BF, tag="xt", name="xt")
            w_big = pp.tile([128, 3 * KT * HPD], BF, tag="w", name="w")

            def load_xt(half):
                nc.sync.dma_start(
                    xt_big[:, half * 4 * S:(half + 1) * 4 * S].rearrange(
                        "p (k n) -> p k n", n=S),
                    blob[OFF_XT + half * 4 * 128 * S:
                         OFF_XT + (half + 1) * 4 * 128 * S].rearrange(
                        "(k p n) -> p k n", k=4, p=128))

            def load_w(i):
                nc.sync.dma_start(
                    w_big[:, i * KT * HPD:(i + 1) * KT * HPD].rearrange(
                        "p (k c) -> p k c", c=HPD),
                    blob[OFF_WQ + i * HID * HPD:OFF_WQ + (i + 1) * HID * HPD]
                    .rearrange("(k p c) -> p k c", k=KT, p=128))

            load_xt(0)
            load_w(2)   # wv
            load_xt(1)
            load_w(0)   # wq
            load_w(1)   # wk
            xt_sb = [xt_big[:, k * S:(k + 1) * S] for k in range(KT)]
            w_sb = {nm: [w_big[:, (i * KT + k) * HPD:(i * KT + k + 1) * HPD]
                         for k in range(KT)]
                    for i, nm in enumerate(("q", "k", "v"))}

            tri_sb = pp.tile([128, 128], BF, tag="tri", name="tri")
            nc.sync.dma_start(tri_sb[:], bview(OFF_TRI, 128, 128))
            bqk_bf = pp.tile([128, 12], BF, tag="bqkb", name="bqkb")
            nc.sync.dma_start(bqk_bf[:], bview(OFF_BQK, 128, 12))
            bqk_sb = pp.tile([128, 12], F32, tag="bqk", name="bqk")
            nc.vector.tensor_copy(bqk_sb[:], bqk_bf[:])
            bv_sb = pp.tile([1, HPD], BF, tag="bv", name="bv")
            nc.sync.dma_start(bv_sb[:], bview(OFF_BV, 1, HPD))
            woe_big = pp.tile([128, HL * HID], BF, tag="woe", name="woe")
            nc.sync.dma_start(
                woe_big.rearrange("p (h j) -> p h j", j=HID),
                blob[OFF_WO:OFF_WO + HL * P * D * HID].rearrange(
                    "(h p j) -> p h j", h=HL, p=128))
            woe_sb = [woe_big[:, h * HID:(h + 1) * HID] for h in range(HL)]

            qt_sb = [pp.tile([128, S], BF, tag=f"qt{h}", name=f"qt{h}") for h in range(HL)]
            kt_sb = [pp.tile([128, S], BF, tag=f"kt{h}", name=f"kt{h}") for h in range(HL)]
            kt2_sb = [pp.tile([128, S], BF, tag=f"kt2{h}", name=f"kt2{h}") for h in range(HL)]
            vaug = [pp.tile([128, G * 65], BF, tag=f"va{j}", name=f"va{j}") for j in range(S // 128)]
            ot2 = [pp.tile([128, S], BF, tag=f"ot2{h}", name=f"ot2{h}") for h in range(HL)]
            oav = [[pp.tile([64, 512], BF, tag=f"oav{In}_{i}", name=f"oav{In}_{i}")
                    for i in range(G)] for In in range(NT)]
            # denominator rows live at partitions {0,32,64,96} of two tiles
            # (DVE writes must be 32-partition aligned); memset so the unused
            # lanes reciprocate 1.0 rather than garbage.
            den = [[pp.tile([128, 512], BF, tag=f"den{In}_{t}", name=f"den{In}_{t}")
                    for t in range(2)] for In in range(NT)]
            for In in range(NT):
                for t in range(2):
                    nc.gpsimd.memset(den[In][t][:], 1.0)

            def emit_qk(mt, nt):
                # Q and K projections for head mt, seq window nt.
                for nm in ("q", "k"):
                    bcol = 0 if nm == "q" else 4
                    acc = ps1.tile([128, 512], F32, tag="b1", name="acc")
                    for k in range(KT):
                        nc.tensor.matmul(
                            acc[:], w_sb[nm][k][:, mt * 128:(mt + 1) * 128],
                            xt_sb[k][:, nt * 512:(nt + 1) * 512],
                            start=(k == 0), stop=(k == KT - 1))
                    dst = qt_sb[mt] if nm == "q" else kt_sb[mt]
                    sl = slice(nt * 512, (nt + 1) * 512)
                    nc.vector.tensor_scalar_add(
                        dst[:, sl], acc[:], bqk_sb[:, bcol + mt:bcol + mt + 1])
                    if nm == "k":
                        # kt2 = kt with partition halves swapped; copy in
                        # SBUF (2x mode, and the PSUM acc is freed sooner)
                        nc.vector.tensor_copy(kt2_sb[mt][0:64, sl],
                                              kt_sb[mt][64:128, sl])
                        nc.vector.tensor_copy(kt2_sb[mt][64:128, sl],
                                              kt_sb[mt][0:64, sl])

            def emit_v(jt):
                # V projection for key block jt -> vaug (v columns + ones row).
                v3 = vaug[jt].rearrange("p (g e) -> p g e", e=65)
                nc.gpsimd.memset(v3[:, :, 64:65], 1.0)
                acc = ps1.tile([128, 512], F32, tag="b1", name="acc")
                for k in range(KT):
                    nc.tensor.matmul(
                        acc[:], xt_sb[k][:, jt * 128:(jt + 1) * 128],
                        w_sb["v"][k][:], start=(k == 0), stop=False)
                nc.tensor.matmul(acc[:], ones[:, 0:128], bv_sb[:],
                                 start=False, stop=True)
                nc.vector.tensor_copy(
                    v3[:, :, 0:64], acc[:].rearrange("p (g e) -> p g e", e=64))

            def emit_attn(In, h):
                JMAX = 4 * In + 4
                isl = slice(In * 512, (In + 1) * 512)
                avp = [ps1.tile([65, 512], F32, tag="b1", name="av") for _ in range(2)]
                for Jn in range(JMAX):
                    diag = Jn >= 4 * In
                    jsl = slice(Jn * 128, (Jn + 1) * 128)
                    if not diag:
                        # full-width block: macro = [pq0 | pq1] for one pk
                        for pk in range(2):
                            lhsA = (kt_sb[h] if pk == 0 else kt2_sb[h])
                            lhsB = (kt2_sb[h] if pk == 0 else kt_sb[h])
                            mac = psc.tile([128, 1024], F32, tag="sc", name="mac")
                            nc.tensor.matmul(
                                mac[:, 0:512], lhsA[0:64, jsl],
                                qt_sb[h][0:64, isl], start=True, stop=True)
                            nc.tensor.matmul(
                                mac[:, 512:1024], lhsB[64:128, jsl],
                                qt_sb[h][64:128, isl], start=True, stop=True)
                            pt = ppl.tile([128, 1024], BF, tag="p", name="p")
                            nc.scalar.activation(
                                pt[:], mac[:],
                                mybir.ActivationFunctionType.Exp, scale=0.125)
                            g = h * 2 + pk
                            first = (Jn == 0 and pk == 0)
                            nc.tensor.matmul(
                                avp[0][:, 0:512],
                                vaug[Jn][:, g * 65:g * 65 + 65],
                                pt[:, 0:512], start=first, stop=False)
                            nc.tensor.matmul(
                                avp[1][:, 0:512],
                                vaug[Jn][:, g * 65:g * 65 + 65],
                                pt[:, 512:1024], start=first, stop=False)
                        continue
                    # diagonal block: macro per pq packs the legal ranges of
                    # both pk contiguously: [c0:512] = pk0, [512:512+W] = pk1,
                    # so one exp covers exactly [c0:512+W].
                    c0 = 128 * (Jn - 4 * In)
                    W = 512 - c0
                    qsl = slice(In * 512 + c0, (In + 1) * 512)
                    for pq in range(2):
                        l0 = kt_sb[h] if pq == 0 else kt2_sb[h]    # k pseudo 0
                        l1 = kt2_sb[h] if pq == 0 else kt_sb[h]    # k pseudo 1
                        hp = slice(pq * 64, pq * 64 + 64)
                        mac = psc.tile([128, 1024], F32, tag="sc", name="mac")
                        nc.tensor.matmul(
                            mac[:, c0:512], l0[hp, jsl], qt_sb[h][hp, qsl],
                            start=True, stop=True)
                        nc.tensor.matmul(
                            mac[:, 512:512 + W], l1[hp, jsl], qt_sb[h][hp, qsl],
                            start=True, stop=True)
                        pt = ppl.tile([128, 1024], BF, tag="p", name="p")
                        nc.scalar.activation(
                            pt[:, c0:512 + W], mac[:, c0:512 + W],
                            mybir.ActivationFunctionType.Exp, scale=0.125)
                        nc.vector.tensor_mul(
                            pt[:, c0:c0 + 128], pt[:, c0:c0 + 128], tri_sb[:])
                        nc.vector.tensor_mul(
                            pt[:, 512:512 + 128], pt[:, 512:512 + 128], tri_sb[:])
                        g = h * 2
                        # both pk accumulate into avp[pq]
                        nc.tensor.matmul(
                            avp[pq][:, c0:512],
                            vaug[Jn][:, g * 65:g * 65 + 65],
                            pt[:, c0:512],
                            start=(In == 0 and Jn == 0), stop=False)
                        nc.tensor.matmul(
                            avp[pq][:, c0:512],
                            vaug[Jn][:, (g + 1) * 65:(g + 1) * 65 + 65],
                            pt[:, 512:512 + W],
                            start=False, stop=(Jn == JMAX - 1))
                for pq in range(2):
                    idx = h * 2 + pq
                    row = 32 * (idx % 4)
                    nc.vector.tensor_copy(
                        den[In][idx // 4][row:row + 1, :], avp[pq][64:65, :])
                    nc.vector.tensor_copy(oav[In][idx][:], avp[pq][0:64, :])

            def emit_norm(In, t):
                # reciprocal + normalize for (h, pq) pairs idx = 4t .. 4t+3
                rec = sml.tile([128, 512], BF, tag="rec", name="rec")
                with nc.allow_low_precision(reason="softmax recip bf16"):
                    nc.vector.reciprocal(rec[:], den[In][t][:])
                for idx in range(4 * t, 4 * t + 4):
                    h, pq = idx // 2, idx % 2
                    row = 32 * (idx % 4)
                    bcp = ps1.tile([64, 512], F32, tag="b1", name="bcp")
                    nc.tensor.matmul(
                        bcp[:], ones4[row:row + 1, :],
                        rec[row:row + 1, :], start=True, stop=True,
                        tile_position=(row, 0))
                    nc.vector.tensor_mul(
                        ot2[h][pq * 64:(pq + 1) * 64, In * 512:(In + 1) * 512],
                        oav[In][idx][:], bcp[:])

            def emit_outproj(In):
                for mt in range(4 * In, 4 * In + 4):
                    ob = osb.tile([128, 1024], BF, tag="ob", name="ob")
                    for jt in range(HID // 512):
                        op = ps1.tile([128, 512], F32, tag="b1", name="op")
                        for h in range(HL):
                            nc.tensor.matmul(
                                op[:], ot2[h][:, mt * 128:(mt + 1) * 128],
                                woe_sb[h][:, jt * 512:(jt + 1) * 512],
                                start=(h == 0), stop=(h == HL - 1))
                        nc.vector.tensor_copy(ob[:, jt * 512:(jt + 1) * 512], op[:])
                    nc.sync.dma_start(out[mt * 128:(mt + 1) * 128, :], ob[:])

            # Interleaved emission: attention for a head starts as soon as its
            # q/k (window slice) and the first key blocks' v are ready, so the
            # ACT exp stream ramps early and projections fill PE stalls.
            for jt in range(4):
                emit_v(jt)
            emit_qk(0, 0)
            emit_qk(0, 1)
            emit_qk(1, 0)
            emit_qk(1, 1)
            emit_attn(0, 0)
            emit_qk(2, 0)
            emit_qk(2, 1)
            emit_attn(0, 1)
            emit_qk(3, 0)
            emit_qk(3, 1)
            emit_attn(0, 2)
            for jt in range(4, 8):
                emit_v(jt)
            emit_attn(0, 3)
            emit_attn(1, 0)
            emit_norm(0, 0)
            emit_norm(0, 1)
            emit_outproj(0)
            emit_attn(1, 1)
            emit_norm(1, 0)
            emit_attn(1, 2)
            emit_attn(1, 3)
            emit_norm(1, 1)
            emit_outproj(1)
    nc.compile()
    return nc


def _prep(inputs):
    bf = ml_dtypes.bfloat16
    hs = np.asarray(inputs["hidden_states"], np.float32)
    maps = []
    tri = np.triu(np.ones((128, 128), np.float32)).astype(bf)  # tri[r,c]=1 iff c>=r
    eff = {}
    for nm in ("q", "k", "v"):
        W = np.asarray(inputs[f"W{nm}"], np.float32)
        bb = np.asarray(inputs[f"b{nm}"], np.float32)
        al = np.asarray(inputs[f"alpha_{nm}"], np.float32)
        We = np.einsum("mhp,mdc->hpdc", al, W.reshape(H, D, HID))
        be = np.einsum("mhp,md->hpd", al, bb.reshape(H, D))
        eff[nm] = (We, be)
    Wo = np.asarray(inputs["Wo"], np.float32)
    col = np.asarray(inputs["collapse"], np.float32)
    Woe = np.einsum("hp,jhd->hpdj", col, Wo.reshape(HID, H, D))  # (H,P,D,HID)
    for c in range(NCORES):
        b, g = c // 4, c % 4
        hs_sl = slice(g * HL, (g + 1) * HL)
        parts = [np.ascontiguousarray(hs[b].T).astype(bf).reshape(-1)]
        for nm in ("q", "k", "v"):
            We, _ = eff[nm]
            Wslice = We[hs_sl].reshape(HPD, HID)      # (hpd, c)
            parts.append(np.ascontiguousarray(Wslice.T).astype(bf).reshape(-1))
        # (128, 12) per-partition bias columns: bqT | bkT | bk2T
        bq = eff["q"][1][hs_sl].reshape(HL, 128).T    # (128, HL)
        bk = eff["k"][1][hs_sl].reshape(HL, 128).T
        bk2 = np.concatenate([bk[64:128], bk[0:64]], axis=0)
        parts.append(np.concatenate([bq, bk, bk2], axis=1).astype(bf).reshape(-1))
        parts.append(eff["v"][1][hs_sl].reshape(-1).astype(bf))
        parts.append(Woe[hs_sl].reshape(-1).astype(bf))
        parts.append(tri.reshape(-1))
        blob = np.concatenate(parts)
        assert blob.shape[0] == TOT, blob.shape
        maps.append({"blob": blob})
    return maps


def kernel(**inputs):
    global _compiled
    if _compiled is None:
        _compiled = _build()
    maps = _prep(inputs)
    res = run_bass_kernel_spmd(_compiled, maps, core_ids=list(range(NCORES)))
    bo = np.asarray(inputs["bo"], np.float32)
    out = np.zeros((B, S, HID), np.float32)
    for c in range(NCORES):
        out[c // 4] += res.results[c]["o"].astype(np.float32)
    out += bo
    return out
